# revision 25
# baseline (speedup 1.0000x reference)
"""Trainium2 Bass kernel for a 2-layer GAT (nn_AGAEMD problem).

Sharding: layer-1 heads across 8 cores (core h owns head h, full N x N
attention for that head); layer-2 row-sharded (core c owns output rows
[c*512, (c+1)*512)).  Head outputs are combined with ONE bf16 AllReduce
over a flat contiguous payload (Who partials + a ones column + gT rows);
the per-core g_src slice is extracted post-AR with a one-hot selection
matmul (rsel input), avoiding any core-dependent addressing.

Math notes:
 - softmax rows are invariant to any per-column factor, so instead of
   P = exp(leaky(fs_i + fd_j))*adj we compute
   G2 = exp(0.8*relu(s) + 0.2*fd_j)*adj  (= P * exp(-0.2*fs_i)),
   which normalizes to the same attention.  Two equivalent pipelines:
     ACT-form: t1 = Relu(fsb + fd_j), t2 = Exp(0.8*t1 + 0.2*fd_j), mask
     DVE-form: u = A8b * B_j (ts), w = max(u, D_j) (ts), mask
   with A8b = exp(0.8*fs_i) broadcast, B = exp(fd), D = exp(0.2*fd).
 - reciprocals are computed as exp(-ln(x)) on the scalar engine (the
   DVE RECIPROCAL instruction costs ~5.3us regardless of size).
 - elu(x) = max(x,0) - 1 + exp(min(x,0)).
"""

import sys

if "/opt/trn_rl_repo" not in sys.path:
    sys.path.insert(0, "/opt/trn_rl_repo")

import numpy as np
import ml_dtypes

BF = ml_dtypes.bfloat16

# problem dims (hardcoded per spec)
N, F, H, D, C = 4096, 256, 8, 64, 64
CORES = 8
SLOPE = 0.2

import os as _os

# engine-split tunables: #ACT-form tiles (of 32) and #mask ops on gpsimd
ACT1_N = int(_os.environ.get("K_ACT1", "16"))
GPS1_N = int(_os.environ.get("K_GPS1", "0"))
ACT2_N = int(_os.environ.get("K_ACT2", "10"))
GPS2_N = int(_os.environ.get("K_GPS2", "0"))
BUFS = int(_os.environ.get("K_BUFS", "3"))
K_DEBUG = int(_os.environ.get("K_DEBUG", "0"))

_BASS_CACHE = {}


def _spread(k, nt):
    """k tile indices spread evenly over range(nt) (Bresenham)."""
    return {t for t in range(nt) if ((t + 1) * k) // nt > (t * k) // nt}


def _emit(nc, tc, n, shard):
    """Emit the SPMD per-core graph. n = graph size (4096 full), shard = n//8."""
    import concourse.bass as bass
    import concourse.mybir as mybir
    from concourse.masks import make_identity

    bf = mybir.dt.bfloat16
    f32 = mybir.dt.float32
    AF = mybir.ActivationFunctionType
    OP = mybir.AluOpType
    NT = n // 128          # number of 128-row tiles
    NH = n // 2            # split-layout free width
    RG = [list(range(CORES))]
    C1 = C + 1             # who payload row: C cols + ones col

    # ---- dram I/O ----
    xT_d = nc.dram_tensor("xT", [F, n], bf, kind="ExternalInput")
    adjT_d = nc.dram_tensor("adjT", [n, n], bf, kind="ExternalInput")
    adjs_d = nc.dram_tensor("adjs", [n, shard], bf, kind="ExternalInput")
    wh_d = nc.dram_tensor("wh", [F, D], bf, kind="ExternalInput")
    whT_d = nc.dram_tensor("whT", [D, F], bf, kind="ExternalInput")
    a2_d = nc.dram_tensor("a2", [D, 2], bf, kind="ExternalInput")
    wo_d = nc.dram_tensor("wo", [D, C], bf, kind="ExternalInput")
    woT_d = nc.dram_tensor("woT", [C, D], bf, kind="ExternalInput")
    ao2_d = nc.dram_tensor("ao2", [C, 2], bf, kind="ExternalInput")
    rsel_d = nc.dram_tensor("rsel", [CORES, 1], bf, kind="ExternalInput")
    out_d = nc.dram_tensor("out", [shard, C], f32, kind="ExternalOutput")

    # collective bounce buffers, split into two row-halves so each half can
    # AllReduce as soon as its producers finish.  Per-half flat layout:
    # [who rows (n/2, C1) | g_src (n/2) | g_dst (n/2)]
    NHF = n // 2
    HALF = NHF * C1 + 2 * NHF
    FLAT = 2 * HALF
    rs_in = nc.dram_tensor("rs_in", [1, FLAT], bf)
    ag_out = nc.dram_tensor("ag_out", [1, FLAT], bf, addr_space="Shared")

    def _views(tensor):
        who, gs, gd = [], [], []
        for hh in range(2):
            b = hh * HALF
            who.append(tensor[0:1, b:b + NHF * C1].rearrange(
                "one (r c) -> (one r) c", c=C1))
            gs.append(tensor[0:1, b + NHF * C1:b + NHF * C1 + NHF])
            gd.append(tensor[0:1, b + NHF * C1 + NHF:b + HALF])
        return who, gs, gd

    who_w, gs_w, gd_w = _views(rs_in)
    who_r, gs_r, gd_r = _views(ag_out)

    from contextlib import ExitStack

    es = ExitStack()
    pers = es.enter_context(tc.tile_pool(name="pers", bufs=1))
    ppool = es.enter_context(tc.tile_pool(name="psum", bufs=1, space="PSUM"))
    pbig = ppool.tile([128, 4096], f32, name="pbig")

    # ---- prologue: weights ----
    xtp = tc.tile_pool(name="xtp", bufs=1)
    xtpool = xtp.__enter__()
    xt = []
    for k in range(2):
        t = xtpool.tile([128, n], bf, name=f"xt{k}")
        nc.sync.dma_start(t[:], xT_d[k * 128:(k + 1) * 128, :])
        xt.append(t)
    wf = []
    for k in range(2):
        t = pers.tile([128, D + 2], bf, name=f"wf{k}")
        nc.sync.dma_start(t[:, 0:D], wh_d[k * 128:(k + 1) * 128, :])
        wf.append(t)
    whTt = pers.tile([128, F], bf, name="whTt")
    nc.gpsimd.memset(whTt[:], 0.0)
    nc.sync.dma_start(whTt[0:D, :], whT_d[:])
    a2t = pers.tile([128, 2], bf, name="a2t")
    nc.gpsimd.memset(a2t[:], 0.0)
    nc.sync.dma_start(a2t[0:D, :], a2_d[:])
    rselt = pers.tile([CORES, 1], bf, name="rselt")
    nc.sync.dma_start(rselt[:], rsel_d[:])

    # waug = W_h @ a2 : [F, 2] (two 128-row tiles)
    for k in range(2):
        pw = pbig[0:128, k * 512:k * 512 + 2]
        nc.tensor.matmul(pw, lhsT=whTt[:, k * 128:(k + 1) * 128], rhs=a2t[:],
                         start=True, stop=True)
        nc.vector.tensor_copy(wf[k][:, D:D + 2], pw)

    # fsrc row via matmul: fsrc = x @ wsrc -> psum rows, split on partitions 0/32
    n_cc = n // 512
    cpg = max(1, NH // 512)  # 512-chunks per half
    for cc in range(n_cc):
        part = (cc // cpg) * 32  # matmul out base partition must be 0/32/64
        foff = 2048 + 512 * (cc % cpg)
        pr = pbig[part:part + 1, foff:foff + 512]
        for k in range(2):
            nc.tensor.matmul(pr, lhsT=wf[k][:, D:D + 1],
                             rhs=xt[k][:, cc * 512:(cc + 1) * 512],
                             start=(k == 0), stop=(k == 1))
    # copy psum fsrc rows -> sbuf (partition-aligned; rows 0 and 32)
    fr = xtpool.tile([33, NH], f32, name="fr")
    nc.vector.tensor_copy(fr[0:1, :], pbig[0:1, 2048:2048 + NH])
    nc.scalar.activation(fr[32:33, :], pbig[32:33, 2048:2048 + NH], AF.Copy)

    # Whaug = x @ [W | wsrc | wdst] -> per i-tile [128, D+2]
    whl = []
    fsd = pers.tile([128, 2 * NT], f32, name="fsd")
    for it in range(NT):
        pwh = pbig[0:128, (it % 4) * 512:(it % 4) * 512 + D + 2]
        for k in range(2):
            nc.tensor.matmul(pwh, lhsT=xt[k][:, it * 128:(it + 1) * 128],
                             rhs=wf[k][:], start=(k == 0), stop=(k == 1))
        t = pers.tile([128, D + 1], bf, name=f"whl{it}")
        nc.vector.tensor_copy(t[:, 0:D], pwh[:, 0:D])
        nc.gpsimd.memset(t[:, D:D + 1], 1.0)
        nc.vector.tensor_copy(fsd[:, 2 * it:2 * it + 2], pwh[:, D:D + 2])
        whl.append(t)

    # broadcast fsrc to all partitions; A8b = exp(0.8*fs_i).
    # NB: partition_broadcast on HW only reads from partition 0, so the
    # offset source row is first DMA-shifted to partition 0.
    frb = xtpool.tile([33, NH], bf, name="frb")
    nc.vector.tensor_copy(frb[0:1, :], fr[0:1, :])
    nc.vector.tensor_copy(frb[32:33, :], fr[32:33, :])
    frb2 = xtpool.tile([1, NH], bf, name="frb2")
    nc.sync.dma_start(frb2[0:1, :], frb[32:33, :])
    fsb = pers.tile([128, n], bf, name="fsb")
    nc.gpsimd.partition_broadcast(fsb[:, 0:NH], frb[0:1, :])
    nc.gpsimd.partition_broadcast(fsb[:, NH:n], frb2[0:1, :])
    A8b = pers.tile([128, n], bf, name="A8b")
    nc.scalar.activation(A8b[:], fsb[:], AF.Exp, scale=0.8)
    xtp.__exit__(None, None, None)

    # per-partition fd constants: raw fd, 0.2*fd, exp(fd), exp(0.2*fd)
    fsdr = fsd.rearrange("p (t two) -> p t two", two=2)
    fdc = pers.tile([128, NT], f32, name="fdc")
    fd02 = pers.tile([128, NT], f32, name="fd02")
    Bc = pers.tile([128, NT], f32, name="Bc")
    Dc = pers.tile([128, NT], f32, name="Dc")
    fdcr = fdc.rearrange("p (t o) -> p t o", o=1)
    fd02r = fd02.rearrange("p (t o) -> p t o", o=1)
    Bcr = Bc.rearrange("p (t o) -> p t o", o=1)
    Dcr = Dc.rearrange("p (t o) -> p t o", o=1)
    nc.vector.tensor_copy(fdcr[:], fsdr[:, :, 1:2])
    nc.vector.tensor_scalar(out=fd02r[:], in0=fsdr[:, :, 1:2], scalar1=SLOPE,
                            scalar2=None, op0=OP.mult)
    nc.scalar.activation(Bcr[:], fsdr[:, :, 1:2], AF.Exp)
    nc.scalar.activation(Dcr[:], fsdr[:, :, 1:2], AF.Exp, scale=SLOPE)

    # woaug = [Wo_h | Wo_h@ao_src | Wo_h@ao_dst]  [D, C+2], duplicated on
    # partitions 0:64 and 64:128 (matmul requires lhsT/rhs base partitions
    # to match; eluO halves live at 0 and 64)
    woTt = pers.tile([128, D], bf, name="woTt")
    nc.gpsimd.memset(woTt[:], 0.0)
    nc.sync.dma_start(woTt[0:C, :], woT_d[:])
    ao2t = pers.tile([128, 2], bf, name="ao2t")
    nc.gpsimd.memset(ao2t[:], 0.0)
    nc.sync.dma_start(ao2t[0:C, :], ao2_d[:])
    woaug = pers.tile([128, C + 2], bf, name="woaug")
    for half in range(2):
        pwo = pbig[half * 64:half * 64 + D, 0:2]
        nc.tensor.matmul(pwo, lhsT=woTt[:, 0:D], rhs=ao2t[:],
                         start=True, stop=True)
        nc.sync.dma_start(woaug[half * 64:half * 64 + D, 0:C], wo_d[:])
        nc.vector.tensor_copy(woaug[half * 64:half * 64 + D, C:C + 2], pwo)

    I128 = pers.tile([128, 128], f32, name="I128")
    make_identity(nc, I128[:])

    # ---- layer-1 j-loop ----
    l1es = ExitStack()
    adj_pool = l1es.enter_context(tc.tile_pool(name="adj", bufs=BUFS))
    t_pool = l1es.enter_context(tc.tile_pool(name="t1", bufs=BUFS))
    u_pool = l1es.enter_context(tc.tile_pool(name="t2", bufs=BUFS))
    p_pool = l1es.enter_context(tc.tile_pool(name="pp", bufs=BUFS))

    act_set = _spread(ACT1_N, NT)
    # masks to gpsimd: prefer ACT-form tiles (their chains avoid DVE)
    order = [t for t in range(NT) if t in act_set] + \
            [t for t in range(NT) if t not in act_set]
    gps_set = set(order[:GPS1_N])

    # layer-2 adj slice, prefetched mid-loop (DMA has slack there) so the
    # transfer is done before the collective window
    adjs2 = pers.tile([128, NT * shard], bf, name="adjs2")
    adjs2v = adjs2.rearrange("p (t i) -> p t i", t=NT)

    nchunk = n // 512
    for t in range(NT):
        adjt = adj_pool.tile([128, n], bf, tag="adjt")
        eng = nc.sync if t % 2 == 0 else nc.scalar
        eng.dma_start(adjt[:], adjT_d[t * 128:(t + 1) * 128, :])
        if t == 20:
            nc.scalar.dma_start(
                adjs2v[:], adjs_d.ap().rearrange("(t p) i -> p t i", p=128))
        P = p_pool.tile([128, n], bf, tag="P")
        if t in act_set:
            tt1 = t_pool.tile([128, n], bf, tag="tt1")
            nc.scalar.activation(tt1[:], fsb[:], AF.Relu,
                                 bias=fdc[:, t:t + 1])
            uu = u_pool.tile([128, n], bf, tag="uu")
            nc.scalar.activation(uu[:], tt1[:], AF.Exp, scale=0.8,
                                 bias=fd02[:, t:t + 1])
        else:
            tt1 = t_pool.tile([128, n], bf, tag="tt1")
            nc.vector.tensor_scalar(out=tt1[:], in0=A8b[:],
                                    scalar1=Bc[:, t:t + 1],
                                    scalar2=None, op0=OP.mult)
            uu = u_pool.tile([128, n], bf, tag="uu")
            nc.vector.tensor_scalar(out=uu[:], in0=tt1[:],
                                    scalar1=Dc[:, t:t + 1],
                                    scalar2=None, op0=OP.max)
        if t in gps_set:
            nc.gpsimd.tensor_tensor(P[:], uu[:], adjt[:], OP.mult)
        else:
            nc.vector.tensor_tensor(P[:], uu[:], adjt[:], OP.mult)
        for c in range(nchunk):
            nc.tensor.matmul(pbig[0:D + 1, c * 512:(c + 1) * 512],
                             lhsT=whl[t][:], rhs=P[:, c * 512:(c + 1) * 512],
                             start=(t == 0), stop=(t == NT - 1))
    l1es.close()

    # ---- layer-1 epilogue: normalize + elu (split [128, NH] layout) ----
    # psum -> sbuf (partition-aligned compute copies on two engines);
    # transient tiles live in a scoped pool freed before layer-2 prep
    epp = tc.tile_pool(name="epp", bufs=1)
    ep = epp.__enter__()
    o1lo = ep.tile([D + 1, NH], f32, name="o1lo")
    o1hi = ep.tile([D + 1, NH], f32, name="o1hi")
    nc.vector.tensor_copy(o1lo[:], pbig[0:D + 1, 0:NH])
    nc.scalar.activation(o1hi[:], pbig[0:D + 1, NH:n], AF.Copy)
    # sbuf->sbuf DMAs to fold into a [128, NH] split layout
    o1s = ep.tile([128, NH], f32, name="o1s")
    nc.sync.dma_start(o1s[0:D, :], o1lo[0:D, :])
    nc.sync.dma_start(o1s[D:2 * D, :], o1hi[0:D, :])
    # 1/Z via exp(-ln(Z)) on the scalar engine (DVE reciprocal has a
    # ~5.3us fixed cost): shift Z rows to partition 0, invert, broadcast.
    # Lns then Exps batched to avoid activation-table swaps.
    zfa = ep.tile([1, NH], f32, name="zfa")
    zfb = ep.tile([1, NH], f32, name="zfb")
    zla = ep.tile([1, NH], bf, name="zla")
    zlb = ep.tile([1, NH], bf, name="zlb")
    zb = ep.tile([128, NH], bf, name="zb")
    zbx = ep.tile([D, NH], bf, name="zbx")
    nc.sync.dma_start(zfa[0:1, :], o1lo[D:D + 1, :])
    nc.sync.dma_start(zfb[0:1, :], o1hi[D:D + 1, :])
    nc.scalar.activation(zfa[0:1, :], zfa[0:1, :], AF.Ln)
    nc.scalar.activation(zfb[0:1, :], zfb[0:1, :], AF.Ln)
    nc.scalar.activation(zla[0:1, :], zfa[0:1, :], AF.Exp, scale=-1.0)
    nc.scalar.activation(zlb[0:1, :], zfb[0:1, :], AF.Exp, scale=-1.0)
    nc.gpsimd.partition_broadcast(zb[0:D, :], zla[0:1, :])
    nc.gpsimd.partition_broadcast(zbx[0:D, :], zlb[0:1, :])
    nc.sync.dma_start(zb[D:2 * D, :], zbx[0:D, :])
    o1n = ep.tile([128, NH], bf, name="o1n")
    nc.vector.tensor_tensor(o1n[:], o1s[:], zb[:], OP.mult)
    # elu
    mm = ep.tile([128, NH], bf, name="mm")
    nc.vector.tensor_scalar(out=mm[:], in0=o1n[:], scalar1=0.0, scalar2=None,
                            op0=OP.min)
    em = ep.tile([128, NH], bf, name="em")
    nc.scalar.activation(em[:], mm[:], AF.Exp)
    r1 = ep.tile([128, NH], bf, name="r1")
    nc.vector.tensor_scalar(out=r1[:], in0=o1n[:], scalar1=0.0, scalar2=-1.0,
                            op0=OP.max, op1=OP.add)
    eluO = pers.tile([128, NH], bf, name="eluO")
    nc.vector.tensor_tensor(eluO[:], r1[:], em[:], OP.add)

    # partial Who = eluO^T.T @ wo -> [n, C] into gts (col C holds 1/8 so the
    # AllReduce sum yields the ones column used for Z2); DMA to who region
    # out-projection, gT, and the AllReduce all run half-by-half (i-rows
    # 0:2048 then 2048:4096) so the first AR overlaps the second half's
    # compute and the first half's post-AR loads overlap the second AR.
    gts = pers.tile([128, NT, C1], bf, name="gts")
    nc.gpsimd.memset(gts[:, :, C:C + 1], 1.0 / CORES)
    gtt = ep.tile([2, n], bf, name="gtt")
    half_t = NT // 2
    for hh in range(2):
        for it in range(hh * half_t, (hh + 1) * half_t):
            col = (it % half_t) * 128
            pt2 = pbig[0:128, (it % 8) * 512:(it % 8) * 512 + C]
            nc.tensor.matmul(pt2, lhsT=eluO[hh * D:(hh + 1) * D, col:col + 128],
                             rhs=woaug[hh * D:(hh + 1) * D, 0:C],
                             start=True, stop=True)
            if it % 2 == 0:
                nc.vector.tensor_copy(gts[:, it, 0:C], pt2)
            else:
                nc.scalar.activation(gts[:, it, 0:C], pt2, AF.Copy)
            if it % 4 == 3:  # grouped stores: 2 DMA issues per half
                gl = it // 4 - hh * 4
                nc.sync.dma_start(
                    who_w[hh][gl * 512:(gl + 1) * 512, :].rearrange(
                        "(t p) c -> p t c", p=128),
                    gts[:, (it // 4) * 4:(it // 4) * 4 + 4, :])
        # gT rows for this half: out[r, i] = sum_d ao2[d, r]*eluO[d, i]
        for cc in range(hh * cpg, (hh + 1) * cpg):
            col = (cc % cpg) * 512
            pg = pbig[0:2, cc * 512:(cc + 1) * 512]
            nc.tensor.matmul(pg, lhsT=woaug[hh * 64:hh * 64 + D, C:C + 2],
                             rhs=eluO[hh * 64:hh * 64 + D, col:col + 512],
                             start=True, stop=True)
        nc.vector.tensor_copy(gtt[:, hh * NHF:(hh + 1) * NHF],
                              pbig[0:2, hh * NHF:(hh + 1) * NHF])
        nc.sync.dma_start(
            rs_in[0:1, hh * HALF + NHF * C1:(hh + 1) * HALF].rearrange(
                "one (g i) -> (one g) i", i=NHF),
            gtt[:, hh * NHF:(hh + 1) * NHF])
        nc.gpsimd.collective_compute(
            "AllReduce", mybir.AluOpType.add, replica_groups=RG,
            ins=[rs_in[0:1, hh * HALF:(hh + 1) * HALF].rearrange(
                "one (a b) -> (one a) b", a=CORES)],
            outs=[ag_out[0:1, hh * HALF:(hh + 1) * HALF].rearrange(
                "one (a b) -> (one a) b", a=CORES)])
    epp.__exit__(None, None, None)

    # ---- layer-2 prep (half-a loads overlap the half-b AllReduce) ----
    whol = pers.tile([128, NT, C1], bf, name="whol")
    gdc = pers.tile([128, NT], bf, name="gdc")
    gs8 = pers.tile([CORES, shard], bf, name="gs8")
    for hh in range(2):
        nc.sync.dma_start(
            whol[:, hh * half_t:(hh + 1) * half_t, :],
            who_r[hh].rearrange("(t p) c -> p t c", p=128))
        nc.sync.dma_start(
            gdc[:, hh * half_t:(hh + 1) * half_t],
            gd_r[hh].rearrange("one (t p) -> (one p) t", p=128))
        nc.sync.dma_start(
            gs8[hh * 4:(hh + 1) * 4, :],
            gs_r[hh].rearrange("one (a i) -> (one a) i", a=4))
    gdf = pers.tile([128, NT], f32, name="gdf")
    gd02 = pers.tile([128, NT], f32, name="gd02")
    B2c = pers.tile([128, NT], f32, name="B2c")
    D2c = pers.tile([128, NT], f32, name="D2c")
    nc.vector.tensor_copy(gdf[:], gdc[:])
    nc.vector.tensor_scalar(out=gd02[:], in0=gdc[:], scalar1=SLOPE,
                            scalar2=None, op0=OP.mult)
    nc.scalar.activation(B2c[:], gdc[:], AF.Exp)
    nc.scalar.activation(D2c[:], gdc[:], AF.Exp, scale=SLOPE)
    # g_src slice for this core: one-hot rsel @ g_src viewed as [8, shard]
    pgs = pbig[0:1, 3584:3584 + shard]
    nc.tensor.matmul(pgs, lhsT=rselt[:], rhs=gs8[:], start=True, stop=True)
    gsr = pers.tile([1, shard], bf, name="gsr")
    nc.vector.tensor_copy(gsr[:], pgs)
    gsb = pers.tile([128, shard], bf, name="gsb")
    nc.gpsimd.partition_broadcast(gsb[:], gsr[0:1, :])
    A2b = pers.tile([128, shard], bf, name="A2b")
    nc.scalar.activation(A2b[:], gsb[:], AF.Exp, scale=0.8)

    # ---- layer-2 j-loop ----
    t_pool = es.enter_context(tc.tile_pool(name="t1b", bufs=BUFS))
    u_pool = es.enter_context(tc.tile_pool(name="t2b", bufs=BUFS))
    p_pool = es.enter_context(tc.tile_pool(name="ppb", bufs=BUFS))
    act2_set = _spread(ACT2_N, NT)
    order2 = [t for t in range(NT) if t in act2_set] + \
             [t for t in range(NT) if t not in act2_set]
    gps2_set = set(order2[:GPS2_N])
    for t in range(NT):
        adjs = adjs2[:, t * shard:(t + 1) * shard]
        P2 = p_pool.tile([128, shard], bf, tag="P2")
        if t in act2_set:
            q1 = t_pool.tile([128, shard], bf, tag="q1")
            nc.scalar.activation(q1[:], gsb[:], AF.Relu,
                                 bias=gdf[:, t:t + 1])
            q2 = u_pool.tile([128, shard], bf, tag="q2")
            nc.scalar.activation(q2[:], q1[:], AF.Exp, scale=0.8,
                                 bias=gd02[:, t:t + 1])
        else:
            q1 = t_pool.tile([128, shard], bf, tag="q1")
            nc.vector.tensor_scalar(out=q1[:], in0=A2b[:],
                                    scalar1=B2c[:, t:t + 1],
                                    scalar2=None, op0=OP.mult)
            q2 = u_pool.tile([128, shard], bf, tag="q2")
            nc.vector.tensor_scalar(out=q2[:], in0=q1[:],
                                    scalar1=D2c[:, t:t + 1],
                                    scalar2=None, op0=OP.max)
        if t in gps2_set:
            nc.gpsimd.tensor_tensor(P2[:], q2[:], adjs, OP.mult)
        else:
            nc.vector.tensor_tensor(P2[:], q2[:], adjs, OP.mult)
        nc.tensor.matmul(pbig[0:C + 1, 0:shard], lhsT=whol[:, t, :], rhs=P2[:],
                         start=(t == 0), stop=(t == NT - 1))

    if K_DEBUG:
        tap_fsd = nc.dram_tensor("tap_fsd", [128, 2 * NT], f32, kind="ExternalOutput")
        nc.sync.dma_start(tap_fsd.ap(), fsd[:])
        tap_o1lo = nc.dram_tensor("tap_o1lo", [D + 1, NH], f32, kind="ExternalOutput")
        nc.sync.dma_start(tap_o1lo.ap(), o1lo[:])
        tap_eluO = nc.dram_tensor("tap_eluO", [128, NH], bf, kind="ExternalOutput")
        nc.sync.dma_start(tap_eluO.ap(), eluO[:])
        tap_rsin = nc.dram_tensor("tap_rsin", [1, FLAT], bf, kind="ExternalOutput")
        nc.sync.dma_start(tap_rsin.ap(), rs_in.ap())
        tap_ag = nc.dram_tensor("tap_ag", [1, FLAT], bf, kind="ExternalOutput")
        nc.sync.dma_start(tap_ag.ap(), ag_out.ap())
        tap_gsb = nc.dram_tensor("tap_gsb", [128, shard], bf, kind="ExternalOutput")
        nc.sync.dma_start(tap_gsb.ap(), gsb[:])
        tap_gdf = nc.dram_tensor("tap_gdf", [128, NT], f32, kind="ExternalOutput")
        nc.sync.dma_start(tap_gdf.ap(), gdf[:])

    # ---- layer-2 epilogue: transpose, normalize, elu, log_softmax ----
    o2t = pers.tile([C + 1, shard], f32, name="o2t")
    nc.vector.tensor_copy(o2t[:], pbig[0:C + 1, 0:shard])
    if K_DEBUG:
        tap_o2t = nc.dram_tensor("tap_o2t", [C + 1, shard], f32, kind="ExternalOutput")
        nc.sync.dma_start(tap_o2t.ap(), o2t[:])
    # stage-major (all chunks per stage) so same-table ACT ops batch and
    # the engines pipeline across chunks
    nst = (shard + 127) // 128
    ws = [min(128, shard - k * 128) for k in range(nst)]
    ptrs = [pbig[0:ws[k], 512 + k * 512:512 + k * 512 + C + 1]
            for k in range(nst)]
    Tl = lambda nm, c=C: [pers.tile([128, c], f32, name=f"{nm}{k}")
                          for k in range(nst)]
    zr, o2n, m2, e2, r2, el2 = (Tl("zr", 1), Tl("o2n"), Tl("m2"), Tl("e2"),
                                Tl("r2"), Tl("el2"))
    mx, xm, ex, sume, lns, ok = (Tl("mx", 1), Tl("xm"), Tl("ex"),
                                 Tl("sume", 1), Tl("lns", 1), Tl("ok"))
    for k in range(nst):
        nc.tensor.transpose(ptrs[k], o2t[:, k * 128:k * 128 + ws[k]],
                            I128[0:C + 1, 0:C + 1])
    for k in range(nst):
        nc.scalar.activation(zr[k][0:ws[k], :], ptrs[k][:, C:C + 1], AF.Ln)
    for k in range(nst):
        nc.scalar.activation(zr[k][0:ws[k], :], zr[k][0:ws[k], :], AF.Exp,
                             scale=-1.0)
    for k in range(nst):
        w = ws[k]
        nc.vector.tensor_scalar(out=o2n[k][0:w, :], in0=ptrs[k][:, 0:C],
                                scalar1=zr[k][0:w, :], scalar2=None,
                                op0=OP.mult)
        nc.vector.tensor_scalar(out=m2[k][0:w, :], in0=o2n[k][0:w, :],
                                scalar1=0.0, scalar2=None, op0=OP.min)
    for k in range(nst):
        nc.scalar.activation(e2[k][0:ws[k], :], m2[k][0:ws[k], :], AF.Exp)
    for k in range(nst):
        w = ws[k]
        nc.vector.tensor_scalar(out=r2[k][0:w, :], in0=o2n[k][0:w, :],
                                scalar1=0.0, scalar2=-1.0, op0=OP.max,
                                op1=OP.add)
        nc.vector.tensor_tensor(el2[k][0:w, :], r2[k][0:w, :], e2[k][0:w, :],
                                OP.add)
        nc.vector.tensor_reduce(mx[k][0:w, :], el2[k][0:w, :],
                                mybir.AxisListType.X, OP.max)
        nc.vector.tensor_scalar(out=xm[k][0:w, :], in0=el2[k][0:w, :],
                                scalar1=mx[k][0:w, :], scalar2=None,
                                op0=OP.subtract)
    for k in range(nst):
        nc.scalar.activation(ex[k][0:ws[k], :], xm[k][0:ws[k], :], AF.Exp,
                             accum_out=sume[k][0:ws[k], :])
    for k in range(nst):
        nc.scalar.activation(lns[k][0:ws[k], :], sume[k][0:ws[k], :], AF.Ln)
    for k in range(nst):
        w = ws[k]
        nc.vector.tensor_scalar(out=ok[k][0:w, :], in0=xm[k][0:w, :],
                                scalar1=lns[k][0:w, :], scalar2=None,
                                op0=OP.subtract)
        nc.sync.dma_start(out_d[k * 128:k * 128 + w, :], ok[k][0:w, :])

    es.close()


def build(n=N, debug=False):
    from concourse import bacc
    import concourse.tile as tile

    nc = bacc.Bacc("TRN2", target_bir_lowering=False, debug=debug,
                   num_devices=CORES)
    with tile.TileContext(nc) as tc:
        _emit(nc, tc, n, n // CORES)
    nc.compile()
    return nc


def make_in_maps(x, adj, W, a, Wo, ao, n=N):
    """Host-side shard/layout prep -> list of 8 input dicts."""
    shard = n // CORES
    xT = np.ascontiguousarray(x.T).astype(BF)
    adjT = np.ascontiguousarray(adj.T).astype(BF)
    in_maps = []
    for h in range(CORES):
        wh = W[h].astype(BF)
        woh = Wo[h * D:(h + 1) * D, :].astype(BF)
        rsel = np.zeros((CORES, 1), dtype=BF)
        rsel[h, 0] = 1.0
        in_maps.append({
            "xT": xT,
            "adjT": adjT,
            "adjs": np.ascontiguousarray(adjT[:, h * shard:(h + 1) * shard]),
            "wh": wh,
            "whT": np.ascontiguousarray(wh.T),
            "a2": np.ascontiguousarray(np.stack([a[h, :D], a[h, D:]], axis=1)).astype(BF),
            "wo": woh,
            "woT": np.ascontiguousarray(woh.T),
            "ao2": np.ascontiguousarray(np.stack([ao[:C], ao[C:]], axis=1)).astype(BF),
            "rsel": rsel,
        })
    return in_maps


def kernel(x, adj, W, a, Wo, ao):
    from concourse.bass_utils import run_bass_kernel_spmd

    x = np.asarray(x, np.float32)
    adj = np.asarray(adj, np.float32)
    W = np.asarray(W, np.float32)
    a = np.asarray(a, np.float32)
    Wo = np.asarray(Wo, np.float32)
    ao = np.asarray(ao, np.float32)

    if "nc" not in _BASS_CACHE:
        _BASS_CACHE["nc"] = build()
    nc = _BASS_CACHE["nc"]
    in_maps = make_in_maps(x, adj, W, a, Wo, ao)
    r = run_bass_kernel_spmd(nc, in_maps, core_ids=list(range(CORES)))
    out = np.concatenate([r.results[c]["out"] for c in range(CORES)], axis=0)
    return np.asarray(out, np.float32)


# revision 28
# speedup vs baseline: 1.0102x; 1.0102x over previous
"""Trainium2 Bass kernel for a 2-layer GAT (nn_AGAEMD problem).

Sharding: layer-1 heads across 8 cores (core h owns head h, full N x N
attention for that head); layer-2 row-sharded (core c owns output rows
[c*512, (c+1)*512)).  Head outputs are combined with ONE bf16 AllReduce
over a flat contiguous payload (Who partials + a ones column + gT rows);
the per-core g_src slice is extracted post-AR with a one-hot selection
matmul (rsel input), avoiding any core-dependent addressing.

Math notes:
 - softmax rows are invariant to any per-column factor, so instead of
   P = exp(leaky(fs_i + fd_j))*adj we compute
   G2 = exp(0.8*relu(s) + 0.2*fd_j)*adj  (= P * exp(-0.2*fs_i)),
   which normalizes to the same attention.  Two equivalent pipelines:
     ACT-form: t1 = Relu(fsb + fd_j), t2 = Exp(0.8*t1 + 0.2*fd_j), mask
     DVE-form: u = A8b * B_j (ts), w = max(u, D_j) (ts), mask
   with A8b = exp(0.8*fs_i) broadcast, B = exp(fd), D = exp(0.2*fd).
 - reciprocals are computed as exp(-ln(x)) on the scalar engine (the
   DVE RECIPROCAL instruction costs ~5.3us regardless of size).
 - elu(x) = max(x,0) - 1 + exp(min(x,0)).
"""

import sys

if "/opt/trn_rl_repo" not in sys.path:
    sys.path.insert(0, "/opt/trn_rl_repo")

import numpy as np
import ml_dtypes

BF = ml_dtypes.bfloat16

# problem dims (hardcoded per spec)
N, F, H, D, C = 4096, 256, 8, 64, 64
CORES = 8
SLOPE = 0.2

import os as _os

# engine-split tunables: #ACT-form tiles (of 32) and #mask ops on gpsimd
ACT1_N = int(_os.environ.get("K_ACT1", "16"))
GPS1_N = int(_os.environ.get("K_GPS1", "0"))
ACT2_N = int(_os.environ.get("K_ACT2", "10"))
GPS2_N = int(_os.environ.get("K_GPS2", "0"))
BUFS = int(_os.environ.get("K_BUFS", "3"))
K_DEBUG = int(_os.environ.get("K_DEBUG", "0"))

_BASS_CACHE = {}


def _spread(k, nt):
    """k tile indices spread evenly over range(nt) (Bresenham)."""
    return {t for t in range(nt) if ((t + 1) * k) // nt > (t * k) // nt}


def _emit(nc, tc, n, shard):
    """Emit the SPMD per-core graph. n = graph size (4096 full), shard = n//8."""
    import concourse.bass as bass
    import concourse.mybir as mybir
    from concourse.masks import make_identity

    bf = mybir.dt.bfloat16
    f32 = mybir.dt.float32
    AF = mybir.ActivationFunctionType
    OP = mybir.AluOpType
    NT = n // 128          # number of 128-row tiles
    NH = n // 2            # split-layout free width
    RG = [list(range(CORES))]
    C1 = C + 1             # who payload row: C cols + ones col

    # ---- dram I/O ----
    xT_d = nc.dram_tensor("xT", [F, n], bf, kind="ExternalInput")
    adjT_d = nc.dram_tensor("adjT", [n, n], bf, kind="ExternalInput")
    adjs_d = nc.dram_tensor("adjs", [n, shard], bf, kind="ExternalInput")
    wh_d = nc.dram_tensor("wh", [F, D], bf, kind="ExternalInput")
    whT_d = nc.dram_tensor("whT", [D, F], bf, kind="ExternalInput")
    a2_d = nc.dram_tensor("a2", [D, 2], bf, kind="ExternalInput")
    wo_d = nc.dram_tensor("wo", [D, C], bf, kind="ExternalInput")
    woT_d = nc.dram_tensor("woT", [C, D], bf, kind="ExternalInput")
    ao2_d = nc.dram_tensor("ao2", [C, 2], bf, kind="ExternalInput")
    rsel_d = nc.dram_tensor("rsel", [CORES, 1], bf, kind="ExternalInput")
    out_d = nc.dram_tensor("out", [shard, C], f32, kind="ExternalOutput")

    # collective bounce buffers, split into two row-halves so each half can
    # AllReduce as soon as its producers finish.  Per-half flat layout:
    # [who rows (n/2, C1) | g_src (n/2) | g_dst (n/2)]
    NHF = n // 2
    HALF = NHF * C1 + 2 * NHF
    FLAT = 2 * HALF
    rs_in = nc.dram_tensor("rs_in", [1, FLAT], bf)
    ag_out = nc.dram_tensor("ag_out", [1, FLAT], bf, addr_space="Shared")

    def _views(tensor):
        who, gs, gd = [], [], []
        for hh in range(2):
            b = hh * HALF
            who.append(tensor[0:1, b:b + NHF * C1].rearrange(
                "one (r c) -> (one r) c", c=C1))
            gs.append(tensor[0:1, b + NHF * C1:b + NHF * C1 + NHF])
            gd.append(tensor[0:1, b + NHF * C1 + NHF:b + HALF])
        return who, gs, gd

    who_w, gs_w, gd_w = _views(rs_in)
    who_r, gs_r, gd_r = _views(ag_out)

    from contextlib import ExitStack

    es = ExitStack()
    pers = es.enter_context(tc.tile_pool(name="pers", bufs=1))
    ppool = es.enter_context(tc.tile_pool(name="psum", bufs=1, space="PSUM"))
    pbig = ppool.tile([128, 4096], f32, name="pbig")

    # ---- prologue: weights ----
    xtp = tc.tile_pool(name="xtp", bufs=1)
    xtpool = xtp.__enter__()
    xt = []
    for k in range(2):
        t = xtpool.tile([128, n], bf, name=f"xt{k}")
        nc.sync.dma_start(t[:], xT_d[k * 128:(k + 1) * 128, :])
        xt.append(t)
    wf = []
    for k in range(2):
        t = pers.tile([128, D + 2], bf, name=f"wf{k}")
        nc.sync.dma_start(t[:, 0:D], wh_d[k * 128:(k + 1) * 128, :])
        wf.append(t)
    whTt = pers.tile([128, F], bf, name="whTt")
    nc.gpsimd.memset(whTt[:], 0.0)
    nc.sync.dma_start(whTt[0:D, :], whT_d[:])
    a2t = pers.tile([128, 2], bf, name="a2t")
    nc.gpsimd.memset(a2t[:], 0.0)
    nc.sync.dma_start(a2t[0:D, :], a2_d[:])
    rselt = pers.tile([CORES, 1], bf, name="rselt")
    nc.sync.dma_start(rselt[:], rsel_d[:])

    # waug = W_h @ a2 : [F, 2] (two 128-row tiles)
    for k in range(2):
        pw = pbig[0:128, k * 512:k * 512 + 2]
        nc.tensor.matmul(pw, lhsT=whTt[:, k * 128:(k + 1) * 128], rhs=a2t[:],
                         start=True, stop=True)
        nc.vector.tensor_copy(wf[k][:, D:D + 2], pw)

    # fsrc row via matmul: fsrc = x @ wsrc -> psum rows, split on partitions 0/32
    n_cc = n // 512
    cpg = max(1, NH // 512)  # 512-chunks per half
    for cc in range(n_cc):
        part = (cc // cpg) * 32  # matmul out base partition must be 0/32/64
        foff = 2048 + 512 * (cc % cpg)
        pr = pbig[part:part + 1, foff:foff + 512]
        for k in range(2):
            nc.tensor.matmul(pr, lhsT=wf[k][:, D:D + 1],
                             rhs=xt[k][:, cc * 512:(cc + 1) * 512],
                             start=(k == 0), stop=(k == 1))
    # copy psum fsrc rows -> sbuf (partition-aligned; rows 0 and 32)
    fr = xtpool.tile([33, NH], f32, name="fr")
    nc.vector.tensor_copy(fr[0:1, :], pbig[0:1, 2048:2048 + NH])
    nc.scalar.activation(fr[32:33, :], pbig[32:33, 2048:2048 + NH], AF.Copy)

    # Whaug = x @ [W | wsrc | wdst] -> per i-tile [128, D+2]
    whl = []
    fsd = pers.tile([128, 2 * NT], f32, name="fsd")
    for it in range(NT):
        pwh = pbig[0:128, (it % 4) * 512:(it % 4) * 512 + D + 2]
        for k in range(2):
            nc.tensor.matmul(pwh, lhsT=xt[k][:, it * 128:(it + 1) * 128],
                             rhs=wf[k][:], start=(k == 0), stop=(k == 1))
        t = pers.tile([128, D + 1], bf, name=f"whl{it}")
        nc.vector.tensor_copy(t[:, 0:D], pwh[:, 0:D])
        nc.gpsimd.memset(t[:, D:D + 1], 1.0)
        nc.vector.tensor_copy(fsd[:, 2 * it:2 * it + 2], pwh[:, D:D + 2])
        whl.append(t)

    # broadcast fsrc to all partitions; A8b = exp(0.8*fs_i).
    # NB: partition_broadcast on HW only reads from partition 0, so the
    # offset source row is first DMA-shifted to partition 0.
    frb = xtpool.tile([33, NH], bf, name="frb")
    nc.vector.tensor_copy(frb[0:1, :], fr[0:1, :])
    nc.vector.tensor_copy(frb[32:33, :], fr[32:33, :])
    frb2 = xtpool.tile([1, NH], bf, name="frb2")
    nc.sync.dma_start(frb2[0:1, :], frb[32:33, :])
    fsb = pers.tile([128, n], bf, name="fsb")
    nc.gpsimd.partition_broadcast(fsb[:, 0:NH], frb[0:1, :])
    nc.gpsimd.partition_broadcast(fsb[:, NH:n], frb2[0:1, :])
    A8b = pers.tile([128, n], bf, name="A8b")
    nc.scalar.activation(A8b[:], fsb[:], AF.Exp, scale=0.8)
    xtp.__exit__(None, None, None)

    # per-partition fd constants: raw fd, 0.2*fd, exp(fd), exp(0.2*fd)
    fsdr = fsd.rearrange("p (t two) -> p t two", two=2)
    fdc = pers.tile([128, NT], f32, name="fdc")
    fd02 = pers.tile([128, NT], f32, name="fd02")
    Bc = pers.tile([128, NT], f32, name="Bc")
    Dc = pers.tile([128, NT], f32, name="Dc")
    fdcr = fdc.rearrange("p (t o) -> p t o", o=1)
    fd02r = fd02.rearrange("p (t o) -> p t o", o=1)
    Bcr = Bc.rearrange("p (t o) -> p t o", o=1)
    Dcr = Dc.rearrange("p (t o) -> p t o", o=1)
    nc.vector.tensor_copy(fdcr[:], fsdr[:, :, 1:2])
    nc.vector.tensor_scalar(out=fd02r[:], in0=fsdr[:, :, 1:2], scalar1=SLOPE,
                            scalar2=None, op0=OP.mult)
    nc.scalar.activation(Bcr[:], fsdr[:, :, 1:2], AF.Exp)
    nc.scalar.activation(Dcr[:], fsdr[:, :, 1:2], AF.Exp, scale=SLOPE)

    # woaug = [Wo_h | Wo_h@ao_src | Wo_h@ao_dst]  [D, C+2], duplicated on
    # partitions 0:64 and 64:128 (matmul requires lhsT/rhs base partitions
    # to match; eluO halves live at 0 and 64)
    woTt = pers.tile([128, D], bf, name="woTt")
    nc.gpsimd.memset(woTt[:], 0.0)
    nc.sync.dma_start(woTt[0:C, :], woT_d[:])
    ao2t = pers.tile([128, 2], bf, name="ao2t")
    nc.gpsimd.memset(ao2t[:], 0.0)
    nc.sync.dma_start(ao2t[0:C, :], ao2_d[:])
    woaug = pers.tile([128, C + 2], bf, name="woaug")
    for half in range(2):
        pwo = pbig[half * 64:half * 64 + D, 0:2]
        nc.tensor.matmul(pwo, lhsT=woTt[:, 0:D], rhs=ao2t[:],
                         start=True, stop=True)
        nc.sync.dma_start(woaug[half * 64:half * 64 + D, 0:C], wo_d[:])
        nc.vector.tensor_copy(woaug[half * 64:half * 64 + D, C:C + 2], pwo)

    I128 = pers.tile([128, 128], f32, name="I128")
    make_identity(nc, I128[:])

    # ---- layer-1 j-loop ----
    l1es = ExitStack()
    adj_pool = l1es.enter_context(tc.tile_pool(name="adj", bufs=BUFS))
    t_pool = l1es.enter_context(tc.tile_pool(name="t1", bufs=BUFS))
    u_pool = l1es.enter_context(tc.tile_pool(name="t2", bufs=BUFS))
    p_pool = l1es.enter_context(tc.tile_pool(name="pp", bufs=BUFS))

    act_set = _spread(ACT1_N, NT)
    # masks to gpsimd: prefer ACT-form tiles (their chains avoid DVE)
    order = [t for t in range(NT) if t in act_set] + \
            [t for t in range(NT) if t not in act_set]
    gps_set = set(order[:GPS1_N])

    # layer-2 adj slice, prefetched mid-loop (DMA has slack there) so the
    # transfer is done before the collective window
    adjs2 = pers.tile([128, NT * shard], bf, name="adjs2")
    adjs2v = adjs2.rearrange("p (t i) -> p t i", t=NT)

    nchunk = n // 512
    for t in range(NT):
        adjt = adj_pool.tile([128, n], bf, tag="adjt")
        eng = nc.sync if t % 2 == 0 else nc.scalar
        eng.dma_start(adjt[:], adjT_d[t * 128:(t + 1) * 128, :])
        if t == 20:
            nc.scalar.dma_start(
                adjs2v[:], adjs_d.ap().rearrange("(t p) i -> p t i", p=128))
        P = p_pool.tile([128, n], bf, tag="P")
        if t in act_set:
            tt1 = t_pool.tile([128, n], bf, tag="tt1")
            nc.scalar.activation(tt1[:], fsb[:], AF.Relu,
                                 bias=fdc[:, t:t + 1])
            uu = u_pool.tile([128, n], bf, tag="uu")
            nc.scalar.activation(uu[:], tt1[:], AF.Exp, scale=0.8,
                                 bias=fd02[:, t:t + 1])
        else:
            tt1 = t_pool.tile([128, n], bf, tag="tt1")
            nc.vector.tensor_scalar(out=tt1[:], in0=A8b[:],
                                    scalar1=Bc[:, t:t + 1],
                                    scalar2=None, op0=OP.mult)
            uu = u_pool.tile([128, n], bf, tag="uu")
            nc.vector.tensor_scalar(out=uu[:], in0=tt1[:],
                                    scalar1=Dc[:, t:t + 1],
                                    scalar2=None, op0=OP.max)
        if t in gps_set:
            nc.gpsimd.tensor_tensor(P[:], uu[:], adjt[:], OP.mult)
        else:
            nc.vector.tensor_tensor(P[:], uu[:], adjt[:], OP.mult)
        for c in range(nchunk):
            nc.tensor.matmul(pbig[0:D + 1, c * 512:(c + 1) * 512],
                             lhsT=whl[t][:], rhs=P[:, c * 512:(c + 1) * 512],
                             start=(t == 0), stop=(t == NT - 1))
    l1es.close()

    # ---- layer-1 epilogue: normalize + elu (split [128, NH] layout) ----
    # psum -> sbuf drains in bf16, then 1/Z via exp(-ln(Z)) on the scalar
    # engine (DVE reciprocal has a ~5.3us fixed cost).  Everything runs
    # column-split (2 chunks of NH/2) so chunk 0's elu/out-projection
    # overlaps chunk 1's drain/normalize; transient tiles live in a scoped
    # pool freed before layer-2 prep.
    epp = tc.tile_pool(name="epp", bufs=1)
    ep = epp.__enter__()
    o1lo = ep.tile([D + 1, NH], bf, name="o1lo")
    o1hi = ep.tile([D + 1, NH], bf, name="o1hi")
    o1s = ep.tile([128, NH], bf, name="o1s")
    zfa = ep.tile([1, NH], bf, name="zfa")
    zfb = ep.tile([1, NH], bf, name="zfb")
    lna = ep.tile([1, NH], f32, name="lna")
    lnb = ep.tile([1, NH], f32, name="lnb")
    zla = ep.tile([1, NH], bf, name="zla")
    zlb = ep.tile([1, NH], bf, name="zlb")
    zb = ep.tile([128, NH], bf, name="zb")
    zbx = ep.tile([D, NH], bf, name="zbx")
    o1n = ep.tile([128, NH], bf, name="o1n")
    mm = ep.tile([128, NH], bf, name="mm")
    em = ep.tile([128, NH], bf, name="em")
    r1 = ep.tile([128, NH], bf, name="r1")
    eluO = pers.tile([128, NH], bf, name="eluO")
    QH = NH // 2
    for q in range(2):
        s = slice(q * QH, (q + 1) * QH)
        nc.vector.tensor_copy(o1lo[:, s], pbig[0:D + 1, q * QH:(q + 1) * QH])
        nc.scalar.activation(o1hi[:, s], pbig[0:D + 1, NH + q * QH:NH + (q + 1) * QH],
                             AF.Copy)
        nc.sync.dma_start(o1s[0:D, s], o1lo[0:D, s])
        nc.sync.dma_start(o1s[D:2 * D, s], o1hi[0:D, s])
        nc.sync.dma_start(zfa[0:1, s], o1lo[D:D + 1, s])
        nc.sync.dma_start(zfb[0:1, s], o1hi[D:D + 1, s])
        nc.scalar.activation(lna[0:1, s], zfa[0:1, s], AF.Ln)
        nc.scalar.activation(lnb[0:1, s], zfb[0:1, s], AF.Ln)
        nc.scalar.activation(zla[0:1, s], lna[0:1, s], AF.Exp, scale=-1.0)
        nc.scalar.activation(zlb[0:1, s], lnb[0:1, s], AF.Exp, scale=-1.0)
        nc.gpsimd.partition_broadcast(zb[0:D, s], zla[0:1, s])
        nc.gpsimd.partition_broadcast(zbx[0:D, s], zlb[0:1, s])
        nc.sync.dma_start(zb[D:2 * D, s], zbx[0:D, s])
        nc.vector.tensor_tensor(o1n[:, s], o1s[:, s], zb[:, s], OP.mult)
        # elu
        nc.vector.tensor_scalar(out=mm[:, s], in0=o1n[:, s], scalar1=0.0,
                                scalar2=None, op0=OP.min)
        nc.scalar.activation(em[:, s], mm[:, s], AF.Exp)
        nc.vector.tensor_scalar(out=r1[:, s], in0=o1n[:, s], scalar1=0.0,
                                scalar2=-1.0, op0=OP.max, op1=OP.add)
        nc.vector.tensor_tensor(eluO[:, s], r1[:, s], em[:, s], OP.add)

    # partial Who = eluO^T.T @ wo -> [n, C] into gts (col C holds 1/8 so the
    # AllReduce sum yields the ones column used for Z2); DMA to who region
    # out-projection, gT, and the AllReduce all run half-by-half (i-rows
    # 0:2048 then 2048:4096) so the first AR overlaps the second half's
    # compute and the first half's post-AR loads overlap the second AR.
    gts = pers.tile([128, NT, C1], bf, name="gts")
    nc.gpsimd.memset(gts[:, :, C:C + 1], 1.0 / CORES)
    gtt = ep.tile([2, n], bf, name="gtt")
    half_t = NT // 2
    for hh in range(2):
        for it in range(hh * half_t, (hh + 1) * half_t):
            col = (it % half_t) * 128
            pt2 = pbig[0:128, (it % 8) * 512:(it % 8) * 512 + C]
            nc.tensor.matmul(pt2, lhsT=eluO[hh * D:(hh + 1) * D, col:col + 128],
                             rhs=woaug[hh * D:(hh + 1) * D, 0:C],
                             start=True, stop=True)
            if it % 2 == 0:
                nc.vector.tensor_copy(gts[:, it, 0:C], pt2)
            else:
                nc.scalar.activation(gts[:, it, 0:C], pt2, AF.Copy)
            if it % 4 == 3:  # grouped stores: 2 DMA issues per half
                gl = it // 4 - hh * 4
                nc.sync.dma_start(
                    who_w[hh][gl * 512:(gl + 1) * 512, :].rearrange(
                        "(t p) c -> p t c", p=128),
                    gts[:, (it // 4) * 4:(it // 4) * 4 + 4, :])
        # gT rows for this half: out[r, i] = sum_d ao2[d, r]*eluO[d, i]
        for cc in range(hh * cpg, (hh + 1) * cpg):
            col = (cc % cpg) * 512
            pg = pbig[0:2, cc * 512:(cc + 1) * 512]
            nc.tensor.matmul(pg, lhsT=woaug[hh * 64:hh * 64 + D, C:C + 2],
                             rhs=eluO[hh * 64:hh * 64 + D, col:col + 512],
                             start=True, stop=True)
        nc.vector.tensor_copy(gtt[:, hh * NHF:(hh + 1) * NHF],
                              pbig[0:2, hh * NHF:(hh + 1) * NHF])
        nc.sync.dma_start(
            rs_in[0:1, hh * HALF + NHF * C1:(hh + 1) * HALF].rearrange(
                "one (g i) -> (one g) i", i=NHF),
            gtt[:, hh * NHF:(hh + 1) * NHF])
    epp.__exit__(None, None, None)

    # single AllReduce (a split-in-two pipeline was tried and lost: each cc
    # op pays its own multi-core rendezvous and they serialize)
    nc.gpsimd.collective_compute(
        "AllReduce", mybir.AluOpType.add, replica_groups=RG,
        ins=[rs_in.ap().rearrange("one (a b) -> (one a) b", a=CORES)],
        outs=[ag_out.ap().rearrange("one (a b) -> (one a) b", a=CORES)])

    # ---- layer-2 prep (half-a loads overlap the half-b AllReduce) ----
    whol = pers.tile([128, NT, C1], bf, name="whol")
    gdc = pers.tile([128, NT], bf, name="gdc")
    gs8 = pers.tile([CORES, shard], bf, name="gs8")
    for hh in range(2):
        nc.sync.dma_start(
            whol[:, hh * half_t:(hh + 1) * half_t, :],
            who_r[hh].rearrange("(t p) c -> p t c", p=128))
        nc.sync.dma_start(
            gdc[:, hh * half_t:(hh + 1) * half_t],
            gd_r[hh].rearrange("one (t p) -> (one p) t", p=128))
        nc.sync.dma_start(
            gs8[hh * 4:(hh + 1) * 4, :],
            gs_r[hh].rearrange("one (a i) -> (one a) i", a=4))
    gdf = pers.tile([128, NT], f32, name="gdf")
    gd02 = pers.tile([128, NT], f32, name="gd02")
    B2c = pers.tile([128, NT], f32, name="B2c")
    D2c = pers.tile([128, NT], f32, name="D2c")
    nc.vector.tensor_copy(gdf[:], gdc[:])
    nc.vector.tensor_scalar(out=gd02[:], in0=gdc[:], scalar1=SLOPE,
                            scalar2=None, op0=OP.mult)
    nc.scalar.activation(B2c[:], gdc[:], AF.Exp)
    nc.scalar.activation(D2c[:], gdc[:], AF.Exp, scale=SLOPE)
    # g_src slice for this core: one-hot rsel @ g_src viewed as [8, shard]
    pgs = pbig[0:1, 3584:3584 + shard]
    nc.tensor.matmul(pgs, lhsT=rselt[:], rhs=gs8[:], start=True, stop=True)
    gsr = pers.tile([1, shard], bf, name="gsr")
    nc.vector.tensor_copy(gsr[:], pgs)
    gsb = pers.tile([128, shard], bf, name="gsb")
    nc.gpsimd.partition_broadcast(gsb[:], gsr[0:1, :])
    A2b = pers.tile([128, shard], bf, name="A2b")
    nc.scalar.activation(A2b[:], gsb[:], AF.Exp, scale=0.8)

    # ---- layer-2 j-loop ----
    t_pool = es.enter_context(tc.tile_pool(name="t1b", bufs=BUFS))
    u_pool = es.enter_context(tc.tile_pool(name="t2b", bufs=BUFS))
    p_pool = es.enter_context(tc.tile_pool(name="ppb", bufs=BUFS))
    act2_set = _spread(ACT2_N, NT)
    order2 = [t for t in range(NT) if t in act2_set] + \
             [t for t in range(NT) if t not in act2_set]
    gps2_set = set(order2[:GPS2_N])
    for t in range(NT):
        adjs = adjs2[:, t * shard:(t + 1) * shard]
        P2 = p_pool.tile([128, shard], bf, tag="P2")
        if t in act2_set:
            q1 = t_pool.tile([128, shard], bf, tag="q1")
            nc.scalar.activation(q1[:], gsb[:], AF.Relu,
                                 bias=gdf[:, t:t + 1])
            q2 = u_pool.tile([128, shard], bf, tag="q2")
            nc.scalar.activation(q2[:], q1[:], AF.Exp, scale=0.8,
                                 bias=gd02[:, t:t + 1])
        else:
            q1 = t_pool.tile([128, shard], bf, tag="q1")
            nc.vector.tensor_scalar(out=q1[:], in0=A2b[:],
                                    scalar1=B2c[:, t:t + 1],
                                    scalar2=None, op0=OP.mult)
            q2 = u_pool.tile([128, shard], bf, tag="q2")
            nc.vector.tensor_scalar(out=q2[:], in0=q1[:],
                                    scalar1=D2c[:, t:t + 1],
                                    scalar2=None, op0=OP.max)
        if t in gps2_set:
            nc.gpsimd.tensor_tensor(P2[:], q2[:], adjs, OP.mult)
        else:
            nc.vector.tensor_tensor(P2[:], q2[:], adjs, OP.mult)
        nc.tensor.matmul(pbig[0:C + 1, 0:shard], lhsT=whol[:, t, :], rhs=P2[:],
                         start=(t == 0), stop=(t == NT - 1))

    if K_DEBUG:
        tap_fsd = nc.dram_tensor("tap_fsd", [128, 2 * NT], f32, kind="ExternalOutput")
        nc.sync.dma_start(tap_fsd.ap(), fsd[:])
        tap_o1lo = nc.dram_tensor("tap_o1lo", [D + 1, NH], f32, kind="ExternalOutput")
        nc.sync.dma_start(tap_o1lo.ap(), o1lo[:])
        tap_eluO = nc.dram_tensor("tap_eluO", [128, NH], bf, kind="ExternalOutput")
        nc.sync.dma_start(tap_eluO.ap(), eluO[:])
        tap_rsin = nc.dram_tensor("tap_rsin", [1, FLAT], bf, kind="ExternalOutput")
        nc.sync.dma_start(tap_rsin.ap(), rs_in.ap())
        tap_ag = nc.dram_tensor("tap_ag", [1, FLAT], bf, kind="ExternalOutput")
        nc.sync.dma_start(tap_ag.ap(), ag_out.ap())
        tap_gsb = nc.dram_tensor("tap_gsb", [128, shard], bf, kind="ExternalOutput")
        nc.sync.dma_start(tap_gsb.ap(), gsb[:])
        tap_gdf = nc.dram_tensor("tap_gdf", [128, NT], f32, kind="ExternalOutput")
        nc.sync.dma_start(tap_gdf.ap(), gdf[:])

    # ---- layer-2 epilogue: transpose, normalize, elu, log_softmax ----
    o2t = pers.tile([C + 1, shard], f32, name="o2t")
    nc.vector.tensor_copy(o2t[:], pbig[0:C + 1, 0:shard])
    if K_DEBUG:
        tap_o2t = nc.dram_tensor("tap_o2t", [C + 1, shard], f32, kind="ExternalOutput")
        nc.sync.dma_start(tap_o2t.ap(), o2t[:])
    # stage-major (all chunks per stage) so same-table ACT ops batch and
    # the engines pipeline across chunks
    nst = (shard + 127) // 128
    ws = [min(128, shard - k * 128) for k in range(nst)]
    ptrs = [pbig[0:ws[k], 512 + k * 512:512 + k * 512 + C + 1]
            for k in range(nst)]
    Tl = lambda nm, c=C: [pers.tile([128, c], f32, name=f"{nm}{k}")
                          for k in range(nst)]
    zr, o2n, m2, e2, r2, el2 = (Tl("zr", 1), Tl("o2n"), Tl("m2"), Tl("e2"),
                                Tl("r2"), Tl("el2"))
    mx, xm, ex, sume, lns, ok = (Tl("mx", 1), Tl("xm"), Tl("ex"),
                                 Tl("sume", 1), Tl("lns", 1), Tl("ok"))
    for k in range(nst):
        nc.tensor.transpose(ptrs[k], o2t[:, k * 128:k * 128 + ws[k]],
                            I128[0:C + 1, 0:C + 1])
    for k in range(nst):
        nc.scalar.activation(zr[k][0:ws[k], :], ptrs[k][:, C:C + 1], AF.Ln)
    for k in range(nst):
        nc.scalar.activation(zr[k][0:ws[k], :], zr[k][0:ws[k], :], AF.Exp,
                             scale=-1.0)
    for k in range(nst):
        w = ws[k]
        nc.vector.tensor_scalar(out=o2n[k][0:w, :], in0=ptrs[k][:, 0:C],
                                scalar1=zr[k][0:w, :], scalar2=None,
                                op0=OP.mult)
        nc.vector.tensor_scalar(out=m2[k][0:w, :], in0=o2n[k][0:w, :],
                                scalar1=0.0, scalar2=None, op0=OP.min)
    for k in range(nst):
        nc.scalar.activation(e2[k][0:ws[k], :], m2[k][0:ws[k], :], AF.Exp)
    for k in range(nst):
        w = ws[k]
        nc.vector.tensor_scalar(out=r2[k][0:w, :], in0=o2n[k][0:w, :],
                                scalar1=0.0, scalar2=-1.0, op0=OP.max,
                                op1=OP.add)
        nc.vector.tensor_tensor(el2[k][0:w, :], r2[k][0:w, :], e2[k][0:w, :],
                                OP.add)
        nc.vector.tensor_reduce(mx[k][0:w, :], el2[k][0:w, :],
                                mybir.AxisListType.X, OP.max)
        nc.vector.tensor_scalar(out=xm[k][0:w, :], in0=el2[k][0:w, :],
                                scalar1=mx[k][0:w, :], scalar2=None,
                                op0=OP.subtract)
    for k in range(nst):
        nc.scalar.activation(ex[k][0:ws[k], :], xm[k][0:ws[k], :], AF.Exp,
                             accum_out=sume[k][0:ws[k], :])
    for k in range(nst):
        nc.scalar.activation(lns[k][0:ws[k], :], sume[k][0:ws[k], :], AF.Ln)
    for k in range(nst):
        w = ws[k]
        nc.vector.tensor_scalar(out=ok[k][0:w, :], in0=xm[k][0:w, :],
                                scalar1=lns[k][0:w, :], scalar2=None,
                                op0=OP.subtract)
        nc.sync.dma_start(out_d[k * 128:k * 128 + w, :], ok[k][0:w, :])

    es.close()


def build(n=N, debug=False):
    from concourse import bacc
    import concourse.tile as tile

    nc = bacc.Bacc("TRN2", target_bir_lowering=False, debug=debug,
                   num_devices=CORES)
    with tile.TileContext(nc) as tc:
        _emit(nc, tc, n, n // CORES)
    nc.compile()
    return nc


def make_in_maps(x, adj, W, a, Wo, ao, n=N):
    """Host-side shard/layout prep -> list of 8 input dicts."""
    shard = n // CORES
    xT = np.ascontiguousarray(x.T).astype(BF)
    adjT = np.ascontiguousarray(adj.T).astype(BF)
    in_maps = []
    for h in range(CORES):
        wh = W[h].astype(BF)
        woh = Wo[h * D:(h + 1) * D, :].astype(BF)
        rsel = np.zeros((CORES, 1), dtype=BF)
        rsel[h, 0] = 1.0
        in_maps.append({
            "xT": xT,
            "adjT": adjT,
            "adjs": np.ascontiguousarray(adjT[:, h * shard:(h + 1) * shard]),
            "wh": wh,
            "whT": np.ascontiguousarray(wh.T),
            "a2": np.ascontiguousarray(np.stack([a[h, :D], a[h, D:]], axis=1)).astype(BF),
            "wo": woh,
            "woT": np.ascontiguousarray(woh.T),
            "ao2": np.ascontiguousarray(np.stack([ao[:C], ao[C:]], axis=1)).astype(BF),
            "rsel": rsel,
        })
    return in_maps


def kernel(x, adj, W, a, Wo, ao):
    from concourse.bass_utils import run_bass_kernel_spmd

    x = np.asarray(x, np.float32)
    adj = np.asarray(adj, np.float32)
    W = np.asarray(W, np.float32)
    a = np.asarray(a, np.float32)
    Wo = np.asarray(Wo, np.float32)
    ao = np.asarray(ao, np.float32)

    if "nc" not in _BASS_CACHE:
        _BASS_CACHE["nc"] = build()
    nc = _BASS_CACHE["nc"]
    in_maps = make_in_maps(x, adj, W, a, Wo, ao)
    r = run_bass_kernel_spmd(nc, in_maps, core_ids=list(range(CORES)))
    out = np.concatenate([r.results[c]["out"] for c in range(CORES)], axis=0)
    return np.asarray(out, np.float32)


# revision 34
# speedup vs baseline: 1.1531x; 1.1415x over previous
"""Trainium2 Bass kernel for a 2-layer GAT (nn_AGAEMD problem).

Sharding: layer-1 heads across 8 cores (core h owns head h, full N x N
attention for that head); layer-2 row-sharded (core c owns output rows
[c*512, (c+1)*512)).  Head outputs are combined with ONE bf16 AllReduce
over a flat contiguous payload (Who partials + a ones column + gT rows);
the per-core g_src slice is extracted post-AR with a one-hot selection
matmul (rsel input), avoiding any core-dependent addressing.

Math notes:
 - softmax rows are invariant to any per-column factor, so instead of
   P = exp(leaky(fs_i + fd_j))*adj we compute
   G2 = exp(0.8*relu(s) + 0.2*fd_j)*adj  (= P * exp(-0.2*fs_i)),
   which normalizes to the same attention.  Two equivalent pipelines:
     ACT-form: t1 = Relu(fsb + fd_j), t2 = Exp(0.8*t1 + 0.2*fd_j), mask
     DVE-form: u = A8b * B_j (ts), w = max(u, D_j) (ts), mask
   with A8b = exp(0.8*fs_i) broadcast, B = exp(fd), D = exp(0.2*fd).
 - reciprocals are computed as exp(-ln(x)) on the scalar engine (the
   DVE RECIPROCAL instruction costs ~5.3us regardless of size).
 - elu(x) = max(x,0) - 1 + exp(min(x,0)).
"""

import sys

if "/opt/trn_rl_repo" not in sys.path:
    sys.path.insert(0, "/opt/trn_rl_repo")

import numpy as np
import ml_dtypes

BF = ml_dtypes.bfloat16

# problem dims (hardcoded per spec)
N, F, H, D, C = 4096, 256, 8, 64, 64
CORES = 8
SLOPE = 0.2

import os as _os

# engine-split tunables: #ACT-form tiles (of 32) and #mask ops on gpsimd
ACT1_N = int(_os.environ.get("K_ACT1", "16"))
GPS1_N = int(_os.environ.get("K_GPS1", "0"))
ACT2_N = int(_os.environ.get("K_ACT2", "10"))
GPS2_N = int(_os.environ.get("K_GPS2", "0"))
BUFS = int(_os.environ.get("K_BUFS", "3"))
K_DEBUG = int(_os.environ.get("K_DEBUG", "0"))

_BASS_CACHE = {}


def _spread(k, nt):
    """k tile indices spread evenly over range(nt) (Bresenham)."""
    return {t for t in range(nt) if ((t + 1) * k) // nt > (t * k) // nt}


def _emit(nc, tc, n, shard):
    """Emit the SPMD per-core graph. n = graph size (4096 full), shard = n//8."""
    import concourse.bass as bass
    import concourse.mybir as mybir
    from concourse.masks import make_identity

    bf = mybir.dt.bfloat16
    f32 = mybir.dt.float32
    AF = mybir.ActivationFunctionType
    OP = mybir.AluOpType
    NT = n // 128          # number of 128-row tiles
    NH = n // 2            # split-layout free width
    RG = [list(range(CORES))]
    C1 = C + 1             # who payload row: C cols + ones col

    # ---- dram I/O ----
    xT_d = nc.dram_tensor("xT", [F, n], bf, kind="ExternalInput")
    adjT_d = nc.dram_tensor("adjT", [n, n], bf, kind="ExternalInput")
    adjs_d = nc.dram_tensor("adjs", [n, shard], bf, kind="ExternalInput")
    wh_d = nc.dram_tensor("wh", [F, D], bf, kind="ExternalInput")
    whT_d = nc.dram_tensor("whT", [D, F], bf, kind="ExternalInput")
    a2_d = nc.dram_tensor("a2", [D, 2], bf, kind="ExternalInput")
    wo_d = nc.dram_tensor("wo", [D, C], bf, kind="ExternalInput")
    woT_d = nc.dram_tensor("woT", [C, D], bf, kind="ExternalInput")
    ao2_d = nc.dram_tensor("ao2", [C, 2], bf, kind="ExternalInput")
    rsel_d = nc.dram_tensor("rsel", [CORES, 1], bf, kind="ExternalInput")
    out_d = nc.dram_tensor("out", [shard, C], f32, kind="ExternalOutput")

    # collective bounce buffers, split into two row-halves so each half can
    # AllReduce as soon as its producers finish.  Per-half flat layout:
    # [who rows (n/2, C1) | g_src (n/2) | g_dst (n/2)]
    NHF = n // 2
    HALF = NHF * C1 + 2 * NHF
    FLAT = 2 * HALF
    rs_in = nc.dram_tensor("rs_in", [1, FLAT], bf)
    ag_out = nc.dram_tensor("ag_out", [1, FLAT], bf, addr_space="Shared")

    def _views(tensor):
        who, gs, gd = [], [], []
        for hh in range(2):
            b = hh * HALF
            who.append(tensor[0:1, b:b + NHF * C1].rearrange(
                "one (r c) -> (one r) c", c=C1))
            gs.append(tensor[0:1, b + NHF * C1:b + NHF * C1 + NHF])
            gd.append(tensor[0:1, b + NHF * C1 + NHF:b + HALF])
        return who, gs, gd

    who_w, gs_w, gd_w = _views(rs_in)
    who_r, gs_r, gd_r = _views(ag_out)

    from contextlib import ExitStack

    es = ExitStack()
    pers = es.enter_context(tc.tile_pool(name="pers", bufs=1))
    ppool = es.enter_context(tc.tile_pool(name="psum", bufs=1, space="PSUM"))
    pbig = ppool.tile([128, 4096], f32, name="pbig")

    # ---- prologue: weights ----
    xtp = tc.tile_pool(name="xtp", bufs=1)
    xtpool = xtp.__enter__()
    xt = []
    for k in range(2):
        t = xtpool.tile([128, n], bf, name=f"xt{k}")
        nc.sync.dma_start(t[:], xT_d[k * 128:(k + 1) * 128, :])
        xt.append(t)
    wf = []
    for k in range(2):
        t = pers.tile([128, D + 2], bf, name=f"wf{k}")
        nc.sync.dma_start(t[:, 0:D], wh_d[k * 128:(k + 1) * 128, :])
        wf.append(t)
    whTt = pers.tile([128, F], bf, name="whTt")
    nc.gpsimd.memset(whTt[:], 0.0)
    nc.sync.dma_start(whTt[0:D, :], whT_d[:])
    a2t = pers.tile([128, 2], bf, name="a2t")
    nc.gpsimd.memset(a2t[:], 0.0)
    nc.sync.dma_start(a2t[0:D, :], a2_d[:])
    rselt = pers.tile([CORES, 1], bf, name="rselt")
    nc.sync.dma_start(rselt[:], rsel_d[:])

    # waug = W_h @ a2 : [F, 2] (two 128-row tiles)
    for k in range(2):
        pw = pbig[0:128, k * 512:k * 512 + 2]
        nc.tensor.matmul(pw, lhsT=whTt[:, k * 128:(k + 1) * 128], rhs=a2t[:],
                         start=True, stop=True)
        nc.vector.tensor_copy(wf[k][:, D:D + 2], pw)

    # fsrc row via matmul: fsrc = x @ wsrc -> psum rows, split on partitions 0/32
    n_cc = n // 512
    cpg = max(1, NH // 512)  # 512-chunks per half
    for cc in range(n_cc):
        part = (cc // cpg) * 32  # matmul out base partition must be 0/32/64
        foff = 2048 + 512 * (cc % cpg)
        pr = pbig[part:part + 1, foff:foff + 512]
        for k in range(2):
            nc.tensor.matmul(pr, lhsT=wf[k][:, D:D + 1],
                             rhs=xt[k][:, cc * 512:(cc + 1) * 512],
                             start=(k == 0), stop=(k == 1))
    # copy psum fsrc rows -> sbuf (partition-aligned; rows 0 and 32)
    fr = xtpool.tile([33, NH], f32, name="fr")
    nc.vector.tensor_copy(fr[0:1, :], pbig[0:1, 2048:2048 + NH])
    nc.scalar.activation(fr[32:33, :], pbig[32:33, 2048:2048 + NH], AF.Copy)

    # Whaug = x @ [W | wsrc | wdst] -> per i-tile [128, D+2].  The factor
    # exp(0.2*fd_j) of G2 is folded into the weights (and the Z column), so
    # the j-loop only needs exp(0.8*relu(s)) = max(A8b*B8_j, 1).
    whl = []
    fsd = pers.tile([128, 2 * NT], f32, name="fsd")
    Dc = pers.tile([128, NT], f32, name="Dc")
    for it in range(NT):
        pwh = pbig[0:128, (it % 4) * 512:(it % 4) * 512 + D + 2]
        for k in range(2):
            nc.tensor.matmul(pwh, lhsT=xt[k][:, it * 128:(it + 1) * 128],
                             rhs=wf[k][:], start=(k == 0), stop=(k == 1))
        nc.scalar.activation(Dc[:, it:it + 1], pwh[:, D + 1:D + 2], AF.Exp,
                             scale=SLOPE)
        t = pers.tile([128, D + 1], bf, name=f"whl{it}")
        nc.vector.tensor_scalar(out=t[:, 0:D], in0=pwh[:, 0:D],
                                scalar1=Dc[:, it:it + 1], scalar2=None,
                                op0=OP.mult)
        nc.vector.tensor_copy(t[:, D:D + 1], Dc[:, it:it + 1])
        nc.vector.tensor_copy(fsd[:, 2 * it:2 * it + 2], pwh[:, D:D + 2])
        whl.append(t)

    # broadcast fsrc to all partitions; A8b = exp(0.8*fs_i).
    # NB: partition_broadcast on HW only reads from partition 0, so the
    # offset source row is first DMA-shifted to partition 0.
    frb = xtpool.tile([33, NH], bf, name="frb")
    nc.vector.tensor_copy(frb[0:1, :], fr[0:1, :])
    nc.vector.tensor_copy(frb[32:33, :], fr[32:33, :])
    frb2 = xtpool.tile([1, NH], bf, name="frb2")
    nc.sync.dma_start(frb2[0:1, :], frb[32:33, :])
    fsb = pers.tile([128, n], bf, name="fsb")
    nc.gpsimd.partition_broadcast(fsb[:, 0:NH], frb[0:1, :])
    nc.gpsimd.partition_broadcast(fsb[:, NH:n], frb2[0:1, :])
    A8b = pers.tile([128, n], bf, name="A8b")
    nc.scalar.activation(A8b[:], fsb[:], AF.Exp, scale=0.8)
    xtp.__exit__(None, None, None)

    # per-partition fd constants: raw fd (ACT-form bias), exp(0.8*fd)
    fsdr = fsd.rearrange("p (t two) -> p t two", two=2)
    fdc = pers.tile([128, NT], f32, name="fdc")
    B8 = pers.tile([128, NT], f32, name="B8")
    fdcr = fdc.rearrange("p (t o) -> p t o", o=1)
    B8r = B8.rearrange("p (t o) -> p t o", o=1)
    nc.vector.tensor_copy(fdcr[:], fsdr[:, :, 1:2])
    nc.scalar.activation(B8r[:], fsdr[:, :, 1:2], AF.Exp, scale=0.8)

    # woaug = [Wo_h | Wo_h@ao_src | Wo_h@ao_dst]  [D, C+2], duplicated on
    # partitions 0:64 and 64:128 (matmul requires lhsT/rhs base partitions
    # to match; eluO halves live at 0 and 64)
    woTt = pers.tile([128, D], bf, name="woTt")
    nc.gpsimd.memset(woTt[:], 0.0)
    nc.sync.dma_start(woTt[0:C, :], woT_d[:])
    ao2t = pers.tile([128, 2], bf, name="ao2t")
    nc.gpsimd.memset(ao2t[:], 0.0)
    nc.sync.dma_start(ao2t[0:C, :], ao2_d[:])
    woaug = pers.tile([128, C + 2], bf, name="woaug")
    for half in range(2):
        pwo = pbig[half * 64:half * 64 + D, 0:2]
        nc.tensor.matmul(pwo, lhsT=woTt[:, 0:D], rhs=ao2t[:],
                         start=True, stop=True)
        nc.sync.dma_start(woaug[half * 64:half * 64 + D, 0:C], wo_d[:])
        nc.vector.tensor_copy(woaug[half * 64:half * 64 + D, C:C + 2], pwo)

    I128 = pers.tile([128, 128], f32, name="I128")
    make_identity(nc, I128[:])

    # ---- layer-1 j-loop ----
    l1es = ExitStack()
    adj_pool = l1es.enter_context(tc.tile_pool(name="adj", bufs=BUFS))
    t_pool = l1es.enter_context(tc.tile_pool(name="t1", bufs=BUFS))
    u_pool = l1es.enter_context(tc.tile_pool(name="t2", bufs=BUFS))
    p_pool = l1es.enter_context(tc.tile_pool(name="pp", bufs=BUFS))

    act_set = _spread(ACT1_N, NT)
    # masks to gpsimd: prefer ACT-form tiles (their chains avoid DVE)
    order = [t for t in range(NT) if t in act_set] + \
            [t for t in range(NT) if t not in act_set]
    gps_set = set(order[:GPS1_N])

    # layer-2 adj slice, prefetched mid-loop (DMA has slack there) so the
    # transfer is done before the collective window
    adjs2 = pers.tile([128, NT * shard], bf, name="adjs2")
    adjs2v = adjs2.rearrange("p (t i) -> p t i", t=NT)

    nchunk = n // 512
    for t in range(NT):
        adjt = adj_pool.tile([128, n], bf, tag="adjt")
        eng = nc.sync if t % 2 == 0 else nc.scalar
        eng.dma_start(adjt[:], adjT_d[t * 128:(t + 1) * 128, :])
        if t == 20:
            nc.scalar.dma_start(
                adjs2v[:], adjs_d.ap().rearrange("(t p) i -> p t i", p=128))
        P = p_pool.tile([128, n], bf, tag="P")
        if t in act_set:
            tt1 = t_pool.tile([128, n], bf, tag="tt1")
            nc.scalar.activation(tt1[:], fsb[:], AF.Relu,
                                 bias=fdc[:, t:t + 1])
            uu = u_pool.tile([128, n], bf, tag="uu")
            nc.scalar.activation(uu[:], tt1[:], AF.Exp, scale=0.8)
        else:
            uu = u_pool.tile([128, n], bf, tag="uu")
            nc.vector.tensor_scalar(out=uu[:], in0=A8b[:],
                                    scalar1=B8[:, t:t + 1], scalar2=1.0,
                                    op0=OP.mult, op1=OP.max)
        if t in gps_set:
            nc.gpsimd.tensor_tensor(P[:], uu[:], adjt[:], OP.mult)
        else:
            nc.vector.tensor_tensor(P[:], uu[:], adjt[:], OP.mult)
        for c in range(nchunk):
            nc.tensor.matmul(pbig[0:D + 1, c * 512:(c + 1) * 512],
                             lhsT=whl[t][:], rhs=P[:, c * 512:(c + 1) * 512],
                             start=(t == 0), stop=(t == NT - 1))
    l1es.close()

    # ---- layer-1 epilogue: normalize + elu (split [128, NH] layout) ----
    # psum -> sbuf drains in bf16, then 1/Z via exp(-ln(Z)) on the scalar
    # engine (DVE reciprocal has a ~5.3us fixed cost).  Everything runs
    # column-split (2 chunks of NH/2) so chunk 0's elu/out-projection
    # overlaps chunk 1's drain/normalize; transient tiles live in a scoped
    # pool freed before layer-2 prep.
    epp = tc.tile_pool(name="epp", bufs=1)
    ep = epp.__enter__()
    o1lo = ep.tile([D + 1, NH], bf, name="o1lo")
    o1hi = ep.tile([D + 1, NH], bf, name="o1hi")
    o1s = ep.tile([128, NH], bf, name="o1s")
    zfa = ep.tile([1, NH], bf, name="zfa")
    zfb = ep.tile([1, NH], bf, name="zfb")
    lna = ep.tile([1, NH], f32, name="lna")
    lnb = ep.tile([1, NH], f32, name="lnb")
    zla = ep.tile([1, NH], bf, name="zla")
    zlb = ep.tile([1, NH], bf, name="zlb")
    zb = ep.tile([128, NH], bf, name="zb")
    zbx = ep.tile([D, NH], bf, name="zbx")
    o1n = ep.tile([128, NH], bf, name="o1n")
    mm = ep.tile([128, NH], bf, name="mm")
    em = ep.tile([128, NH], bf, name="em")
    r1 = ep.tile([128, NH], bf, name="r1")
    eluO = pers.tile([128, NH], bf, name="eluO")
    QH = NH // 2
    for q in range(2):
        s = slice(q * QH, (q + 1) * QH)
        nc.vector.tensor_copy(o1lo[:, s], pbig[0:D + 1, q * QH:(q + 1) * QH])
        nc.scalar.activation(o1hi[:, s], pbig[0:D + 1, NH + q * QH:NH + (q + 1) * QH],
                             AF.Copy)
        nc.sync.dma_start(o1s[0:D, s], o1lo[0:D, s])
        nc.sync.dma_start(o1s[D:2 * D, s], o1hi[0:D, s])
        nc.sync.dma_start(zfa[0:1, s], o1lo[D:D + 1, s])
        nc.sync.dma_start(zfb[0:1, s], o1hi[D:D + 1, s])
        nc.scalar.activation(lna[0:1, s], zfa[0:1, s], AF.Ln)
        nc.scalar.activation(lnb[0:1, s], zfb[0:1, s], AF.Ln)
        nc.scalar.activation(zla[0:1, s], lna[0:1, s], AF.Exp, scale=-1.0)
        nc.scalar.activation(zlb[0:1, s], lnb[0:1, s], AF.Exp, scale=-1.0)
        nc.gpsimd.partition_broadcast(zb[0:D, s], zla[0:1, s])
        nc.gpsimd.partition_broadcast(zbx[0:D, s], zlb[0:1, s])
        nc.sync.dma_start(zb[D:2 * D, s], zbx[0:D, s])
        nc.vector.tensor_tensor(o1n[:, s], o1s[:, s], zb[:, s], OP.mult)
        # elu
        nc.vector.tensor_scalar(out=mm[:, s], in0=o1n[:, s], scalar1=0.0,
                                scalar2=None, op0=OP.min)
        nc.scalar.activation(em[:, s], mm[:, s], AF.Exp)
        nc.vector.tensor_scalar(out=r1[:, s], in0=o1n[:, s], scalar1=0.0,
                                scalar2=-1.0, op0=OP.max, op1=OP.add)
        nc.vector.tensor_tensor(eluO[:, s], r1[:, s], em[:, s], OP.add)

    # partial Who = eluO^T.T @ wo -> [n, C] into gts (col C holds 1/8 so the
    # AllReduce sum yields the ones column used for Z2); DMA to who region
    # out-projection, gT, and the AllReduce all run half-by-half (i-rows
    # 0:2048 then 2048:4096) so the first AR overlaps the second half's
    # compute and the first half's post-AR loads overlap the second AR.
    gts = pers.tile([128, NT, C1], bf, name="gts")
    nc.gpsimd.memset(gts[:, :, C:C + 1], 1.0 / CORES)
    gtt = ep.tile([2, n], bf, name="gtt")
    half_t = NT // 2
    for hh in range(2):
        for it in range(hh * half_t, (hh + 1) * half_t):
            col = (it % half_t) * 128
            pt2 = pbig[0:128, (it % 8) * 512:(it % 8) * 512 + C]
            nc.tensor.matmul(pt2, lhsT=eluO[hh * D:(hh + 1) * D, col:col + 128],
                             rhs=woaug[hh * D:(hh + 1) * D, 0:C],
                             start=True, stop=True)
            if it % 2 == 0:
                nc.vector.tensor_copy(gts[:, it, 0:C], pt2)
            else:
                nc.scalar.activation(gts[:, it, 0:C], pt2, AF.Copy)
            if it % 4 == 3:  # grouped stores: 2 DMA issues per half
                gl = it // 4 - hh * 4
                nc.sync.dma_start(
                    who_w[hh][gl * 512:(gl + 1) * 512, :].rearrange(
                        "(t p) c -> p t c", p=128),
                    gts[:, (it // 4) * 4:(it // 4) * 4 + 4, :])
        # gT rows for this half: out[r, i] = sum_d ao2[d, r]*eluO[d, i]
        for cc in range(hh * cpg, (hh + 1) * cpg):
            col = (cc % cpg) * 512
            pg = pbig[0:2, cc * 512:(cc + 1) * 512]
            nc.tensor.matmul(pg, lhsT=woaug[hh * 64:hh * 64 + D, C:C + 2],
                             rhs=eluO[hh * 64:hh * 64 + D, col:col + 512],
                             start=True, stop=True)
        nc.vector.tensor_copy(gtt[:, hh * NHF:(hh + 1) * NHF],
                              pbig[0:2, hh * NHF:(hh + 1) * NHF])
        nc.sync.dma_start(
            rs_in[0:1, hh * HALF + NHF * C1:(hh + 1) * HALF].rearrange(
                "one (g i) -> (one g) i", i=NHF),
            gtt[:, hh * NHF:(hh + 1) * NHF])
    epp.__exit__(None, None, None)

    # single AllReduce (a split-in-two pipeline was tried and lost: each cc
    # op pays its own multi-core rendezvous and they serialize)
    nc.gpsimd.collective_compute(
        "AllReduce", mybir.AluOpType.add, replica_groups=RG,
        ins=[rs_in.ap().rearrange("one (a b) -> (one a) b", a=CORES)],
        outs=[ag_out.ap().rearrange("one (a b) -> (one a) b", a=CORES)])

    # ---- layer-2 prep (half-a loads overlap the half-b AllReduce) ----
    whol = pers.tile([128, NT, C1], bf, name="whol")
    gdc = pers.tile([128, NT], bf, name="gdc")
    gs8 = pers.tile([CORES, shard], bf, name="gs8")
    for hh in range(2):
        nc.sync.dma_start(
            whol[:, hh * half_t:(hh + 1) * half_t, :],
            who_r[hh].rearrange("(t p) c -> p t c", p=128))
        nc.sync.dma_start(
            gdc[:, hh * half_t:(hh + 1) * half_t],
            gd_r[hh].rearrange("one (t p) -> (one p) t", p=128))
        nc.sync.dma_start(
            gs8[hh * 4:(hh + 1) * 4, :],
            gs_r[hh].rearrange("one (a i) -> (one a) i", a=4))
    gdf = pers.tile([128, NT], f32, name="gdf")
    B28 = pers.tile([128, NT], f32, name="B28")
    D2c = pers.tile([128, NT], f32, name="D2c")
    nc.vector.tensor_copy(gdf[:], gdc[:])
    nc.scalar.activation(B28[:], gdc[:], AF.Exp, scale=0.8)
    nc.scalar.activation(D2c[:], gdc[:], AF.Exp, scale=SLOPE)
    # fold exp(0.2*gd_j) into the layer-2 weights + Z column
    for t in range(NT):
        nc.vector.tensor_scalar(out=whol[:, t, :], in0=whol[:, t, :],
                                scalar1=D2c[:, t:t + 1], scalar2=None,
                                op0=OP.mult)
    # g_src slice for this core: one-hot rsel @ g_src viewed as [8, shard]
    pgs = pbig[0:1, 3584:3584 + shard]
    nc.tensor.matmul(pgs, lhsT=rselt[:], rhs=gs8[:], start=True, stop=True)
    gsr = pers.tile([1, shard], bf, name="gsr")
    nc.vector.tensor_copy(gsr[:], pgs)
    gsb = pers.tile([128, shard], bf, name="gsb")
    nc.gpsimd.partition_broadcast(gsb[:], gsr[0:1, :])
    A2b = pers.tile([128, shard], bf, name="A2b")
    nc.scalar.activation(A2b[:], gsb[:], AF.Exp, scale=0.8)

    # ---- layer-2 j-loop ----
    t_pool = es.enter_context(tc.tile_pool(name="t1b", bufs=BUFS))
    u_pool = es.enter_context(tc.tile_pool(name="t2b", bufs=BUFS))
    p_pool = es.enter_context(tc.tile_pool(name="ppb", bufs=BUFS))
    act2_set = _spread(ACT2_N, NT)
    order2 = [t for t in range(NT) if t in act2_set] + \
             [t for t in range(NT) if t not in act2_set]
    gps2_set = set(order2[:GPS2_N])
    for t in range(NT):
        adjs = adjs2[:, t * shard:(t + 1) * shard]
        P2 = p_pool.tile([128, shard], bf, tag="P2")
        if t in act2_set:
            q1 = t_pool.tile([128, shard], bf, tag="q1")
            nc.scalar.activation(q1[:], gsb[:], AF.Relu,
                                 bias=gdf[:, t:t + 1])
            q2 = u_pool.tile([128, shard], bf, tag="q2")
            nc.scalar.activation(q2[:], q1[:], AF.Exp, scale=0.8)
        else:
            q2 = u_pool.tile([128, shard], bf, tag="q2")
            nc.vector.tensor_scalar(out=q2[:], in0=A2b[:],
                                    scalar1=B28[:, t:t + 1], scalar2=1.0,
                                    op0=OP.mult, op1=OP.max)
        if t in gps2_set:
            nc.gpsimd.tensor_tensor(P2[:], q2[:], adjs, OP.mult)
        else:
            nc.vector.tensor_tensor(P2[:], q2[:], adjs, OP.mult)
        nc.tensor.matmul(pbig[0:C + 1, 0:shard], lhsT=whol[:, t, :], rhs=P2[:],
                         start=(t == 0), stop=(t == NT - 1))

    if K_DEBUG:
        tap_fsd = nc.dram_tensor("tap_fsd", [128, 2 * NT], f32, kind="ExternalOutput")
        nc.sync.dma_start(tap_fsd.ap(), fsd[:])
        tap_o1lo = nc.dram_tensor("tap_o1lo", [D + 1, NH], f32, kind="ExternalOutput")
        nc.sync.dma_start(tap_o1lo.ap(), o1lo[:])
        tap_eluO = nc.dram_tensor("tap_eluO", [128, NH], bf, kind="ExternalOutput")
        nc.sync.dma_start(tap_eluO.ap(), eluO[:])
        tap_rsin = nc.dram_tensor("tap_rsin", [1, FLAT], bf, kind="ExternalOutput")
        nc.sync.dma_start(tap_rsin.ap(), rs_in.ap())
        tap_ag = nc.dram_tensor("tap_ag", [1, FLAT], bf, kind="ExternalOutput")
        nc.sync.dma_start(tap_ag.ap(), ag_out.ap())
        tap_gsb = nc.dram_tensor("tap_gsb", [128, shard], bf, kind="ExternalOutput")
        nc.sync.dma_start(tap_gsb.ap(), gsb[:])
        tap_gdf = nc.dram_tensor("tap_gdf", [128, NT], f32, kind="ExternalOutput")
        nc.sync.dma_start(tap_gdf.ap(), gdf[:])

    # ---- layer-2 epilogue: transpose, normalize, elu, log_softmax ----
    o2t = pers.tile([C + 1, shard], f32, name="o2t")
    nc.vector.tensor_copy(o2t[:], pbig[0:C + 1, 0:shard])
    if K_DEBUG:
        tap_o2t = nc.dram_tensor("tap_o2t", [C + 1, shard], f32, kind="ExternalOutput")
        nc.sync.dma_start(tap_o2t.ap(), o2t[:])
    # stage-major (all chunks per stage) so same-table ACT ops batch and
    # the engines pipeline across chunks
    nst = (shard + 127) // 128
    ws = [min(128, shard - k * 128) for k in range(nst)]
    ptrs = [pbig[0:ws[k], 512 + k * 512:512 + k * 512 + C + 1]
            for k in range(nst)]
    Tl = lambda nm, c=C: [pers.tile([128, c], f32, name=f"{nm}{k}")
                          for k in range(nst)]
    zr, o2n, m2, e2, r2, el2 = (Tl("zr", 1), Tl("o2n"), Tl("m2"), Tl("e2"),
                                Tl("r2"), Tl("el2"))
    mx, xm, ex, sume, lns, ok = (Tl("mx", 1), Tl("xm"), Tl("ex"),
                                 Tl("sume", 1), Tl("lns", 1), Tl("ok"))
    for k in range(nst):
        nc.tensor.transpose(ptrs[k], o2t[:, k * 128:k * 128 + ws[k]],
                            I128[0:C + 1, 0:C + 1])
    for k in range(nst):
        nc.scalar.activation(zr[k][0:ws[k], :], ptrs[k][:, C:C + 1], AF.Ln)
    for k in range(nst):
        nc.scalar.activation(zr[k][0:ws[k], :], zr[k][0:ws[k], :], AF.Exp,
                             scale=-1.0)
    for k in range(nst):
        w = ws[k]
        nc.vector.tensor_scalar(out=o2n[k][0:w, :], in0=ptrs[k][:, 0:C],
                                scalar1=zr[k][0:w, :], scalar2=None,
                                op0=OP.mult)
        nc.vector.tensor_scalar(out=m2[k][0:w, :], in0=o2n[k][0:w, :],
                                scalar1=0.0, scalar2=None, op0=OP.min)
    for k in range(nst):
        nc.scalar.activation(e2[k][0:ws[k], :], m2[k][0:ws[k], :], AF.Exp)
    for k in range(nst):
        w = ws[k]
        nc.vector.tensor_scalar(out=r2[k][0:w, :], in0=o2n[k][0:w, :],
                                scalar1=0.0, scalar2=-1.0, op0=OP.max,
                                op1=OP.add)
        nc.vector.tensor_tensor(el2[k][0:w, :], r2[k][0:w, :], e2[k][0:w, :],
                                OP.add)
        nc.vector.tensor_reduce(mx[k][0:w, :], el2[k][0:w, :],
                                mybir.AxisListType.X, OP.max)
        nc.vector.tensor_scalar(out=xm[k][0:w, :], in0=el2[k][0:w, :],
                                scalar1=mx[k][0:w, :], scalar2=None,
                                op0=OP.subtract)
    for k in range(nst):
        nc.scalar.activation(ex[k][0:ws[k], :], xm[k][0:ws[k], :], AF.Exp,
                             accum_out=sume[k][0:ws[k], :])
    for k in range(nst):
        nc.scalar.activation(lns[k][0:ws[k], :], sume[k][0:ws[k], :], AF.Ln)
    for k in range(nst):
        w = ws[k]
        nc.vector.tensor_scalar(out=ok[k][0:w, :], in0=xm[k][0:w, :],
                                scalar1=lns[k][0:w, :], scalar2=None,
                                op0=OP.subtract)
        nc.sync.dma_start(out_d[k * 128:k * 128 + w, :], ok[k][0:w, :])

    es.close()


def build(n=N, debug=False):
    from concourse import bacc
    import concourse.tile as tile
    import concourse.hw_specs as hw_specs

    # Steer every activation to the one table set containing exp+ln+relu+
    # copy, so Ln/Exp mixing never swaps tables (each swap is ~1.3us on the
    # scalar engine).  Set ids stay aligned with act_info.json because only
    # the *contents* of the non-preferred entries are blanked for the
    # chooser; positions are untouched.
    KEEP = "natural_log_exp_and_others"
    orig_tables = bacc.get_activation_tables

    def _tables(arch):
        t = orig_tables(arch)
        if KEEP not in t:
            return t
        return {k: (v if k == KEEP else set()) for k, v in t.items()}

    bacc.get_activation_tables = _tables
    hw_specs_orig = hw_specs.get_activation_tables
    hw_specs.get_activation_tables = _tables
    try:
        nc = bacc.Bacc("TRN2", target_bir_lowering=False, debug=debug,
                       num_devices=CORES)
        with tile.TileContext(nc) as tc:
            _emit(nc, tc, n, n // CORES)
        nc.compile()
    finally:
        bacc.get_activation_tables = orig_tables
        hw_specs.get_activation_tables = hw_specs_orig
    return nc


def make_in_maps(x, adj, W, a, Wo, ao, n=N):
    """Host-side shard/layout prep -> list of 8 input dicts."""
    shard = n // CORES
    xT = np.ascontiguousarray(x.T).astype(BF)
    adjT = np.ascontiguousarray(adj.T).astype(BF)
    in_maps = []
    for h in range(CORES):
        wh = W[h].astype(BF)
        woh = Wo[h * D:(h + 1) * D, :].astype(BF)
        rsel = np.zeros((CORES, 1), dtype=BF)
        rsel[h, 0] = 1.0
        in_maps.append({
            "xT": xT,
            "adjT": adjT,
            "adjs": np.ascontiguousarray(adjT[:, h * shard:(h + 1) * shard]),
            "wh": wh,
            "whT": np.ascontiguousarray(wh.T),
            "a2": np.ascontiguousarray(np.stack([a[h, :D], a[h, D:]], axis=1)).astype(BF),
            "wo": woh,
            "woT": np.ascontiguousarray(woh.T),
            "ao2": np.ascontiguousarray(np.stack([ao[:C], ao[C:]], axis=1)).astype(BF),
            "rsel": rsel,
        })
    return in_maps


def kernel(x, adj, W, a, Wo, ao):
    from concourse.bass_utils import run_bass_kernel_spmd

    x = np.asarray(x, np.float32)
    adj = np.asarray(adj, np.float32)
    W = np.asarray(W, np.float32)
    a = np.asarray(a, np.float32)
    Wo = np.asarray(Wo, np.float32)
    ao = np.asarray(ao, np.float32)

    if "nc" not in _BASS_CACHE:
        _BASS_CACHE["nc"] = build()
    nc = _BASS_CACHE["nc"]
    in_maps = make_in_maps(x, adj, W, a, Wo, ao)
    r = run_bass_kernel_spmd(nc, in_maps, core_ids=list(range(CORES)))
    out = np.concatenate([r.results[c]["out"] for c in range(CORES)], axis=0)
    return np.asarray(out, np.float32)
